# revision 1
# baseline (speedup 1.0000x reference)
"""Multi-head attention (decode: 4 new tokens, 4096-token KV cache) on 8
Trainium2 NeuronCores via Bass/Tile.

Sharding: tensor-parallel over heads (16 heads / 8 cores = 2 per core).
Each core emits a partial o_proj output [64, 2048]; the host sums them.

Memory-regime design: the KV cache dominates HBM traffic, so the host
pre-packs it once per (batch, head) into DMA-friendly low-precision
arrays (quantization error budget: observed rel err ~8e-3 vs the 2e-2
gate; scores are tiny, ~N(0, 0.02), so fp8 K/q is safe, while V errors
hit the output linearly so V stays fp16):
  k8[bh]  = [128, 4096] fp8e4m3: K^T in permuted token order (token
            t = 32p + j lives at chunk j, row p -- attention is
            permutation-invariant over tokens; the cache mask, when
            nonzero, is permuted to match)
  v16[bh] = [128, 4096] fp16: V in natural (p, j, d) blocks = the same
            permuted order
Each partition row is contiguous -> fully linear DMAs at 3/8 the bytes
of the f32 original (48 MB/core). The measured per-core HBM rate with
all 8 cores streaming is ~160 GB/s, and the kernel sits ~10 us above
that pure-DMA floor: all compute is hidden behind the cache stream.

Per-core device algorithm:
  - projections qT/kT [hd, bs] and v [bs, hd] on PE (fp16 weights),
    RoPE on DVE in f32, fp16/fp8 copies of q/k for the attention MMs
  - all new-token scores batched into one PSUM tile + one exp (off the
    per-bh critical path)
  - per bh=(batch, head): stream k8/v16 on the sync ring only (the
    scalar/ACT queue runs exp, whose semaphore waits would stall DMA
    triggers queued behind them); 32 fp8 score matmuls (stationary =
    K^T chunk, moving = q8 [128,4]) into one PSUM tile [128, 32*4];
    single masked exp on ACT -> e fp16; 32+1 fp16 V matmuls accumulated
    into PSUM [128, 4] (the +1 is the new-token V); V matmuls lag one
    bh so PE never waits on ACT
  - no max subtraction: scores are q.k/sqrt(128)(+mask); exp underflows
    to zero for masked slots
  - denominators via ones-matmul (replicated across partitions),
    reciprocal, normalize outT, then o_proj (fp16 weights)
"""

import numpy as np

B, S, H, NH, HD, CACHE = 16, 4, 2048, 16, 128, 4096
NCORES = 8
NHL = NH // NCORES          # heads per core
BS = B * S                  # 64
NCH = CACHE // 128          # 32 cache chunks of 128 tokens
BH = B * NHL                # (batch, head) pairs per core
KH = H // 128               # 16 contraction chunks for projections
ROPE_BASE = 10000.0
KVLOOK = 5                  # kv DMA prefetch depth (bufs = KVLOOK + 2)

_CACHE = {}


def _build_nc(cache_mask=False, repeat=1):
    from contextlib import ExitStack

    import concourse.bass as bass
    import concourse.tile as tile
    from concourse import bacc, mybir

    f32 = mybir.dt.float32
    f16 = mybir.dt.float16
    f8 = mybir.dt.float8e4
    AL = mybir.AluOpType
    AX = mybir.AxisListType
    ACT_EXP = mybir.ActivationFunctionType.Exp

    nc = bacc.Bacc("TRN2", target_bir_lowering=False, debug=False,
                   num_devices=NCORES)

    hT = nc.dram_tensor("hT", [H, BS], f16, kind="ExternalInput").ap()
    wqT = nc.dram_tensor("wqT", [H, NHL * HD], f16, kind="ExternalInput").ap()
    wkT = nc.dram_tensor("wkT", [H, NHL * HD], f16, kind="ExternalInput").ap()
    wvT = nc.dram_tensor("wvT", [H, NHL * HD], f16, kind="ExternalInput").ap()
    woT = nc.dram_tensor("woT", [NHL * HD, H], f16, kind="ExternalInput").ap()
    k8 = nc.dram_tensor("k8", [BH, 128, NCH * 128], f8,
                        kind="ExternalInput").ap()
    v16 = nc.dram_tensor("v16", [BH, 128, NCH * 128], f16,
                         kind="ExternalInput").ap()
    maskT = nc.dram_tensor("maskT", [128, B, NCH, S], f32,
                           kind="ExternalInput").ap()
    maskN = nc.dram_tensor("maskN", [S, B * S], f32, kind="ExternalInput").ap()
    cosq = nc.dram_tensor("cosq", [HD, BS], f32, kind="ExternalInput").ap()
    sinq = nc.dram_tensor("sinq", [HD, BS], f32, kind="ExternalInput").ap()
    cosk = nc.dram_tensor("cosk", [HD, BS], f32, kind="ExternalInput").ap()
    sink = nc.dram_tensor("sink", [HD, BS], f32, kind="ExternalInput").ap()
    outp = nc.dram_tensor("outp", [BS, H], f32, kind="ExternalOutput").ap()

    half = HD // 2

    with tile.TileContext(nc) as tc, ExitStack() as top:
        consts = top.enter_context(tc.tile_pool(name="consts", bufs=1))
        stage = top.enter_context(tc.tile_pool(name="stage", bufs=1))

        ones = consts.tile([128, 128], f32)
        nc.vector.memset(ones[:], 1.0)
        ones_h = consts.tile([S, 128], f16)
        nc.vector.memset(ones_h[:], 1.0)

        cq = consts.tile([HD, BS], f32)
        sq = consts.tile([HD, BS], f32)
        ck = consts.tile([HD, BS], f32)
        sk = consts.tile([HD, BS], f32)
        nc.sync.dma_start(cq[:], cosq)
        nc.sync.dma_start(sq[:], sinq)
        nc.sync.dma_start(ck[:], cosk)
        nc.sync.dma_start(sk[:], sink)

        hT_sb = consts.tile([128, KH, BS], f16)
        nc.sync.dma_start(hT_sb[:], hT.rearrange("(p j) n -> p j n", p=128))

        if cache_mask:
            mT_sb = consts.tile([128, B, NCH * S], f32)
            nc.sync.dma_start(mT_sb[:], maskT.rearrange("p b j q -> p b (j q)"))
        mN_sb = consts.tile([S, B, S], f32)
        nc.sync.dma_start(mN_sb[:], maskN.rearrange("t (b q) -> t b q", b=B))

        wo_sb = consts.tile([128, NHL, H], f16)
        nc.scalar.dma_start(wo_sb[:], woT.rearrange("(h p) n -> p h n", p=128))

        # flattened per-(b,h,q) column index: j = h*BS + b*S + q (h-major)
        qT_h = stage.tile([128, NHL, BS], f16)
        qT_8 = stage.tile([128, NHL, BS], f8)
        kT_h = stage.tile([128, NHL, BS], f16)
        v_st = stage.tile([S, B, NHL * HD], f16)    # new-token V [t, b, h*HD+d]
        v_sb = stage.tile([BS, NHL * HD], f16)
        en_all = stage.tile([S, BH * S], f16)       # exp of new-token scoresT
        epart = stage.tile([128, BH * S], f32)      # denominator partials
        oT_all = stage.tile([128, BH * S], f32)     # unnormalized outT
        recip = stage.tile([128, BH * S], f32)      # 1/denominator replicated
        oTn = stage.tile([128, BH * S], f16)        # normalized outT (fp16)

        for _rep in range(repeat):
            _attention_body(nc, tc, tile, mybir, cache_mask, locals())

    nc.compile()
    return nc


def _attention_body(nc, tc, tile, mybir, cache_mask, env):
    from contextlib import ExitStack

    f32 = mybir.dt.float32
    f16 = mybir.dt.float16
    f8 = mybir.dt.float8e4
    AL = mybir.AluOpType
    AX = mybir.AxisListType
    ACT_EXP = mybir.ActivationFunctionType.Exp
    half = HD // 2

    wqT, wkT, wvT = env["wqT"], env["wkT"], env["wvT"]
    k8, v16, outp = env["k8"], env["v16"], env["outp"]
    cq, sq, ck, sk = env["cq"], env["sq"], env["ck"], env["sk"]
    hT_sb, mN_sb, wo_sb = env["hT_sb"], env["mN_sb"], env["wo_sb"]
    mT_sb = env.get("mT_sb")
    ones, ones_h = env["ones"], env["ones_h"]
    qT_h, kT_h, qT_8 = env["qT_h"], env["kT_h"], env["qT_8"]
    v_st, v_sb = env["v_st"], env["v_sb"]
    en_all, epart = env["en_all"], env["epart"]
    oT_all, recip, oTn = env["oT_all"], env["recip"], env["oTn"]

    if True:
        with ExitStack() as p1:
            kvpool = p1.enter_context(tc.tile_pool(name="kv", bufs=KVLOOK + 2))
            kt_t = [None] * BH
            vt_t = [None] * BH
            e_sb = [None] * BH
            po = [None] * BH

            def kv_dma(bh):
                # both halves on the sync ring: the scalar(ACT) queue runs the
                # exp instructions, whose semaphore waits would stall DMA
                # triggers queued behind them
                tk = kvpool.tile([128, NCH, 128], f8, tag="k8")
                nc.sync.dma_start(
                    tk[:], k8[bh].rearrange("p (j d) -> p j d", j=NCH))
                tv = kvpool.tile([128, NCH, 128], f16, tag="v16")
                nc.sync.dma_start(
                    tv[:], v16[bh].rearrange("p (j d) -> p j d", j=NCH))
                kt_t[bh] = tk
                vt_t[bh] = tv

            # prefetch the first kv tiles before anything else so the DMA
            # rings are never idle during the projection phase
            for bh in range(min(KVLOOK + 1, BH)):
                kv_dma(bh)

            # ---- projections + RoPE ----
            with ExitStack() as ph:
                wpool = ph.enter_context(tc.tile_pool(name="wts", bufs=1))
                ppool = ph.enter_context(
                    tc.tile_pool(name="pproj", bufs=1, space="PSUM"))
                tpool = ph.enter_context(tc.tile_pool(name="ropetmp", bufs=2))
                qkpool = ph.enter_context(tc.tile_pool(name="qk32", bufs=2))

                wq_sb = wpool.tile([128, KH, NHL * HD], f16, tag="w")
                nc.scalar.dma_start(
                    wq_sb[:], wqT.rearrange("(p j) n -> p j n", p=128))
                wk_sb = wpool.tile([128, KH, NHL * HD], f16, tag="w2")
                nc.scalar.dma_start(
                    wk_sb[:], wkT.rearrange("(p j) n -> p j n", p=128))
                wv_sb = wpool.tile([128, KH, NHL * HD], f16, tag="w3")
                nc.scalar.dma_start(
                    wv_sb[:], wvT.rearrange("(p j) n -> p j n", p=128))

                def rope(dst, psrc, cos_t, sin_t):
                    # dst = psrc * cos + shift64(psrc) * sin (sin sign-folded)
                    tmp = tpool.tile([128, BS], f32, tag="ropetmp")
                    nc.vector.tensor_tensor(
                        out=tmp[0:half, :], in0=psrc[half:128, :],
                        in1=sin_t[0:half, :], op=AL.mult)
                    nc.vector.tensor_tensor(
                        out=tmp[half:128, :], in0=psrc[0:half, :],
                        in1=sin_t[half:128, :], op=AL.mult)
                    dst32 = qkpool.tile([128, BS], f32, tag="qk32")
                    nc.vector.tensor_tensor(
                        out=dst32[:], in0=psrc[:], in1=cos_t[:], op=AL.mult)
                    nc.vector.tensor_tensor(
                        out=dst32[:], in0=dst32[:], in1=tmp[:], op=AL.add)
                    nc.vector.tensor_copy(dst, dst32[:])

                for h in range(NHL):
                    pq = ppool.tile([128, BS], f32, tag=f"pq{h}")
                    pk = ppool.tile([128, BS], f32, tag=f"pk{h}")
                    for c in range(KH):
                        nc.tensor.matmul(
                            pq[:], lhsT=wq_sb[:, c, h * HD:(h + 1) * HD],
                            rhs=hT_sb[:, c, :], start=(c == 0),
                            stop=(c == KH - 1))
                    for c in range(KH):
                        nc.tensor.matmul(
                            pk[:], lhsT=wk_sb[:, c, h * HD:(h + 1) * HD],
                            rhs=hT_sb[:, c, :], start=(c == 0),
                            stop=(c == KH - 1))
                    rope(qT_h[:, h, :], pq[:], cq, sq)
                    rope(kT_h[:, h, :], pk[:], ck, sk)
                    nc.vector.tensor_copy(qT_8[:, h, :], qT_h[:, h, :])

                pv = ppool.tile([BS, NHL * HD], f32, tag="pv")
                for c in range(KH):
                    nc.tensor.matmul(
                        pv[:], lhsT=hT_sb[:, c, :], rhs=wv_sb[:, c, :],
                        start=(c == 0), stop=(c == KH - 1))
                nc.vector.tensor_copy(v_sb[:], pv[:])
                for b in range(B):
                    nc.gpsimd.dma_start(v_st[:, b, :],
                                        v_sb[b * S:(b + 1) * S, :])

            # ---- fused attention pass over (b, h) ----
            epool = p1.enter_context(tc.tile_pool(name="e", bufs=3))
            s4pool = p1.enter_context(tc.tile_pool(name="s4", bufs=2))
            npool = p1.enter_context(tc.tile_pool(name="ntmp", bufs=2))
            pspool = p1.enter_context(
                tc.tile_pool(name="psc", bufs=3, space="PSUM"))
            popool = p1.enter_context(
                tc.tile_pool(name="po", bufs=3, space="PSUM"))
            pnpool = p1.enter_context(
                tc.tile_pool(name="psn", bufs=1, space="PSUM"))

            # ---- new-token scores, batched: one PSUM tile, one exp ----
            pn_all = pnpool.tile([S, BH * S], f32, tag="pn")
            for bh in range(BH):
                b, h = divmod(bh, NHL)
                col = h * BS + b * S
                nc.tensor.matmul(
                    pn_all[:, col:col + S],
                    lhsT=kT_h[:, h, b * S:(b + 1) * S],
                    rhs=qT_h[:, h, b * S:(b + 1) * S],
                    start=True, stop=True)
            sn_all = npool.tile([S, BH * S], f32, tag="sn")
            for h in range(NHL):
                nc.vector.tensor_tensor(
                    out=sn_all[:, h * BS:(h + 1) * BS],
                    in0=pn_all[:, h * BS:(h + 1) * BS],
                    in1=mN_sb.rearrange("t b q -> t (b q)"), op=AL.add)
            nc.scalar.activation(en_all[:], sn_all[:], ACT_EXP)

            def scores(bh):
                b, h = divmod(bh, NHL)
                col = h * BS + b * S
                qs8 = qT_8[:, h, b * S:(b + 1) * S]
                # cache scores: 32 stationary-K matmuls into one PSUM tile
                ps = pspool.tile([128, NCH, S], f32, tag="ps")
                for j in range(NCH):
                    nc.tensor.matmul(
                        ps[:, j, :], lhsT=kt_t[bh][:, j, :], rhs=qs8,
                        start=True, stop=True)
                e = epool.tile([128, NCH, S], f16, tag="e")
                if cache_mask:
                    s4 = s4pool.tile([128, NCH * S], f32, tag="s4")
                    nc.vector.tensor_tensor(
                        out=s4[:], in0=ps.rearrange("p j q -> p (j q)"),
                        in1=mT_sb[:, b, :], op=AL.add)
                    nc.scalar.activation(
                        e.rearrange("p j q -> p (j q)"), s4[:], ACT_EXP)
                else:
                    nc.scalar.activation(
                        e.rearrange("p j q -> p (j q)"),
                        ps.rearrange("p j q -> p (j q)"), ACT_EXP)
                e_sb[bh] = e
                nc.vector.reduce_sum(
                    epart[:, col:col + S], e.rearrange("p j q -> p q j"),
                    axis=AX.X)

            def vpass(bh):
                b, h = divmod(bh, NHL)
                col = h * BS + b * S
                p = popool.tile([128, S], f32, tag="po")
                for j in range(NCH):
                    nc.tensor.matmul(
                        p[:], lhsT=vt_t[bh][:, j, :], rhs=e_sb[bh][:, j, :],
                        start=(j == 0), stop=False)
                nc.tensor.matmul(
                    p[:], lhsT=v_st[:, b, h * HD:(h + 1) * HD],
                    rhs=en_all[:, col:col + S], start=False, stop=True)
                po[bh] = p
                nc.vector.tensor_copy(oT_all[:, col:col + S], p[:])

            for bh in range(BH):
                scores(bh)
                if bh + KVLOOK + 1 < BH:
                    kv_dma(bh + KVLOOK + 1)
                if bh > 0:
                    vpass(bh - 1)
            vpass(BH - 1)

        # ---- denominators + normalize ----
        with ExitStack() as pd_:
            dpool = pd_.enter_context(
                tc.tile_pool(name="pden", bufs=1, space="PSUM"))
            pd = dpool.tile([128, BH * S], f32)
            nc.tensor.matmul(pd[:], lhsT=ones[:], rhs=epart[:],
                             start=True, stop=False)
            nc.tensor.matmul(pd[:], lhsT=ones_h[:], rhs=en_all[:],
                             start=False, stop=True)
            nc.vector.reciprocal(recip[:], pd[:])
            nc.vector.tensor_tensor(out=oTn[:], in0=oT_all[:],
                                    in1=recip[:], op=AL.mult)

        # ---- o_proj ----
        with ExitStack() as po_:
            opool = po_.enter_context(tc.tile_pool(name="oout", bufs=2))
            oppool = po_.enter_context(
                tc.tile_pool(name="pop", bufs=2, space="PSUM"))
            NBLK = 512
            for nb in range(H // NBLK):
                pout = oppool.tile([BS, NBLK], f32, tag="pout")
                for h in range(NHL):
                    nc.tensor.matmul(
                        pout[:], lhsT=oTn[:, h * BS:(h + 1) * BS],
                        rhs=wo_sb[:, h, nb * NBLK:(nb + 1) * NBLK],
                        start=(h == 0), stop=(h == NHL - 1))
                osb = opool.tile([BS, NBLK], f32, tag="osb")
                nc.vector.tensor_copy(osb[:], pout[:])
                nc.sync.dma_start(outp[:, nb * NBLK:(nb + 1) * NBLK],
                                  osb[:])


def _get_nc(cache_mask=False):
    key = ("nc", cache_mask)
    if key not in _CACHE:
        _CACHE[key] = _build_nc(cache_mask=cache_mask)
    return _CACHE[key]


def _prep_inputs(hidden_states, position_ids, past_key, past_value,
                 attention_mask, Wq, Wk, Wv, Wo):
    """Host-side marshaling: per-core input dicts."""
    f = np.float32
    h16 = np.float16
    hidden = np.asarray(hidden_states, f)
    pos = np.asarray(position_ids)
    pk = np.asarray(past_key, f)
    pv = np.asarray(past_value, f)
    mask = np.asarray(attention_mask, f)
    Wq = np.asarray(Wq, f)
    Wk = np.asarray(Wk, f)
    Wv = np.asarray(Wv, f)
    Wo = np.asarray(Wo, f)

    hT = np.ascontiguousarray(hidden.reshape(BS, H).T.astype(h16))

    posf = pos.reshape(BS).astype(f)
    inv_freq = (1.0 / (ROPE_BASE ** (np.arange(0, HD, 2, dtype=f) / HD))).astype(f)
    ang = posf[:, None] * inv_freq[None, :]          # [BS, 64]
    cos = np.cos(ang).astype(f).T                    # [64, BS]
    sin = np.sin(ang).astype(f).T
    cos_full = np.concatenate([cos, cos], axis=0)    # [128, BS]
    sin_fold = np.concatenate([-sin, sin], axis=0)   # sign-folded
    scale = f(1.0 / np.sqrt(HD))
    cosq = np.ascontiguousarray(cos_full * scale)
    sinq = np.ascontiguousarray(sin_fold * scale)
    cosk = np.ascontiguousarray(cos_full)
    sink = np.ascontiguousarray(sin_fold)

    m = mask[:, 0]                                   # [B, S, TOTAL]
    mc = m[:, :, :CACHE]                             # [B, S, CACHE]
    # kernel token order: cache token t = 32*p + j -> maskT[p, b, j, q]
    maskT = np.ascontiguousarray(
        mc.reshape(B, S, 128, NCH).transpose(2, 0, 3, 1))
    mn = m[:, :, CACHE:]                             # [B, S, S]
    maskN = np.ascontiguousarray(mn.transpose(2, 0, 1).reshape(S, B * S))

    # packed fp16 KV: [BH, 128, 8192] per core; K half is K^T in the
    # permuted token order (chunk j, row p <-> token 32p + j), V half is V
    # in natural (p, j, d) blocks = same permuted order.
    pk5 = pk.reshape(B, NH, 128, NCH, HD)
    pv5 = pv.reshape(B, NH, 128, NCH, HD)

    in_maps = []
    for core in range(NCORES):
        h0 = core * NHL
        rows = slice(h0 * HD, (h0 + NHL) * HD)
        from concourse import mybir as _mybir
        f8np = _mybir.dt.np(_mybir.dt.float8e4)
        ktp = pk5[:, h0:h0 + NHL].transpose(0, 1, 4, 3, 2).astype(f8np)
        vp = pv5[:, h0:h0 + NHL].astype(h16)
        in_maps.append({
            "hT": hT,
            "wqT": np.ascontiguousarray(Wq[rows, :].T.astype(h16)),
            "wkT": np.ascontiguousarray(Wk[rows, :].T.astype(h16)),
            "wvT": np.ascontiguousarray(Wv[rows, :].T.astype(h16)),
            "woT": np.ascontiguousarray(Wo[:, rows].T.astype(h16)),
            "k8": np.ascontiguousarray(ktp.reshape(BH, 128, NCH * HD)),
            "v16": np.ascontiguousarray(vp.reshape(BH, 128, NCH * HD)),
            "maskT": maskT,
            "maskN": maskN,
            "cosq": cosq, "sinq": sinq, "cosk": cosk, "sink": sink,
        })
    return in_maps


def kernel(**inputs):
    from concourse.bass_utils import run_bass_kernel_spmd

    # The cache-region mask is structurally zero for this module (causal mask
    # over tokens that all precede the new ones). Only build the general
    # masked variant if the input actually carries nonzero cache-mask values.
    mc = np.asarray(inputs["attention_mask"], np.float32)[:, 0, :, :CACHE]
    nc = _get_nc(cache_mask=bool(np.any(mc != 0.0)))
    in_maps = _prep_inputs(**inputs)
    res = run_bass_kernel_spmd(nc, in_maps, list(range(NCORES)), trace=False)
    out = np.zeros((BS, H), np.float32)
    for r in res.results:
        out += r["outp"]
    return out.reshape(B, S, H)



# revision 2
# speedup vs baseline: 2.4907x; 2.4907x over previous
"""Multi-head attention (decode: 4 new tokens, 4096-token KV cache) on 8
Trainium2 NeuronCores via Bass/Tile.

Sharding: tensor-parallel over heads (16 heads / 8 cores = 2 per core).
Each core emits a partial o_proj output [64, 2048]; the host sums them.

Memory-regime design: the KV cache dominates HBM traffic, so the host
pre-packs it once per (batch, head) into one fp8 array:
  kv8[bh] = [128, 8192] fp8e4m3: first 4096 columns are K^T in permuted
            token order (token t = 32p + j lives at chunk j, row p --
            attention is permutation-invariant over tokens; the cache
            mask, when nonzero, is permuted to match), last 4096 are V
            in natural (p, j, d) blocks = the same permuted order.
One fully linear 1 MB DMA per (batch, head) = 32 MB/core/iter, vs the
~185 GB/s/core practical HBM rate measured with all 8 cores streaming.

fp8 V alone would add ~3.6% output error (the attention output is a
near-uniform average, so per-element V noise passes straight through).
The same near-uniformity makes the error correctable: the dominant term
of sum_i a_i * eps_i is (mean_i eps_i) * (sum_i a_i), and the host knows
eps = v - fp8(v) exactly. The kernel adds the rank-1 correction
meps[d] * Ecache[q] (Ecache = cache softmax mass, already computed for
the denominators) via one tiny 1-partition matmul per (b,h). Residual
error ~ score_std * eps_std ~ 0.1%.

Per-core device algorithm:
  - projections qT/kT [hd, bs] and v [bs, hd] on PE (fp16 weights,
    loaded once into SBUF), RoPE on DVE in f32, fp16/fp8 copies of q/k
  - all new-token scores batched into one PSUM tile + one exp
  - per bh=(batch, head): stream kv8 on the sync ring only (the
    scalar/ACT queue runs exp, whose semaphore waits would stall DMA
    triggers queued behind them); 32 fp8 score matmuls into one PSUM
    tile [128, 32*4]; single masked exp on ACT -> e fp16; 32+1 matmuls
    (fp8 V stationary x fp16 e moving) accumulated into PSUM [128, 4];
    V matmuls lag one bh so PE never waits on ACT
  - no max subtraction: scores are q.k/sqrt(128)(+mask); exp underflows
    to zero for masked slots
  - denominators via ones-matmul, cache-mass row + V-residual rank-1
    correction, reciprocal, normalize outT, then o_proj (fp16 weights)
"""

import numpy as np

B, S, H, NH, HD, CACHE = 16, 4, 2048, 16, 128, 4096
NCORES = 8
NHL = NH // NCORES          # heads per core
BS = B * S                  # 64
NCH = CACHE // 128          # 32 cache chunks of 128 tokens
BH = B * NHL                # (batch, head) pairs per core
KH = H // 128               # 16 contraction chunks for projections
ROPE_BASE = 10000.0
KVLOOK = 5                  # kv DMA prefetch depth (bufs = KVLOOK + 2)

_CACHE = {}


def _build_nc(cache_mask=False, repeat=1):
    from contextlib import ExitStack

    import concourse.bass as bass
    import concourse.tile as tile
    from concourse import bacc, mybir

    f32 = mybir.dt.float32
    f16 = mybir.dt.float16
    f8 = mybir.dt.float8e4
    AL = mybir.AluOpType
    AX = mybir.AxisListType
    ACT_EXP = mybir.ActivationFunctionType.Exp

    nc = bacc.Bacc("TRN2", target_bir_lowering=False, debug=False,
                   num_devices=NCORES)

    hT = nc.dram_tensor("hT", [H, BS], f16, kind="ExternalInput").ap()
    wqT = nc.dram_tensor("wqT", [H, NHL * HD], f16, kind="ExternalInput").ap()
    wkT = nc.dram_tensor("wkT", [H, NHL * HD], f16, kind="ExternalInput").ap()
    wvT = nc.dram_tensor("wvT", [H, NHL * HD], f16, kind="ExternalInput").ap()
    woT = nc.dram_tensor("woT", [NHL * HD, H], f16, kind="ExternalInput").ap()
    kv8 = nc.dram_tensor("kv8", [BH, 128, 2 * NCH * 128], f8,
                         kind="ExternalInput").ap()
    meps = nc.dram_tensor("meps", [1, BH * HD], f16,
                          kind="ExternalInput").ap()
    maskT = nc.dram_tensor("maskT", [128, B, NCH, S], f32,
                           kind="ExternalInput").ap()
    maskN = nc.dram_tensor("maskN", [S, B * S], f32, kind="ExternalInput").ap()
    cosq = nc.dram_tensor("cosq", [HD, BS], f32, kind="ExternalInput").ap()
    sinq = nc.dram_tensor("sinq", [HD, BS], f32, kind="ExternalInput").ap()
    cosk = nc.dram_tensor("cosk", [HD, BS], f32, kind="ExternalInput").ap()
    sink = nc.dram_tensor("sink", [HD, BS], f32, kind="ExternalInput").ap()
    outp = nc.dram_tensor("outp", [BS, H], f16, kind="ExternalOutput").ap()

    with tile.TileContext(nc) as tc, ExitStack() as top:
        consts = top.enter_context(tc.tile_pool(name="consts", bufs=1))
        stage = top.enter_context(tc.tile_pool(name="stage", bufs=1))

        ones = consts.tile([128, 128], f32)
        nc.vector.memset(ones[:], 1.0)
        ones_h = consts.tile([S, 128], f16)
        nc.vector.memset(ones_h[:], 1.0)

        cq = consts.tile([HD, BS], f32)
        sq = consts.tile([HD, BS], f32)
        ck = consts.tile([HD, BS], f32)
        sk = consts.tile([HD, BS], f32)
        nc.sync.dma_start(cq[:], cosq)
        nc.sync.dma_start(sq[:], sinq)
        nc.sync.dma_start(ck[:], cosk)
        nc.sync.dma_start(sk[:], sink)

        hT_sb = consts.tile([128, KH, BS], f16)
        nc.sync.dma_start(hT_sb[:], hT.rearrange("(p j) n -> p j n", p=128))

        if cache_mask:
            mT_sb = consts.tile([128, B, NCH * S], f32)
            nc.sync.dma_start(mT_sb[:], maskT.rearrange("p b j q -> p b (j q)"))
        mN_sb = consts.tile([S, B, S], f32)
        nc.sync.dma_start(mN_sb[:], maskN.rearrange("t (b q) -> t b q", b=B))

        wo_sb = consts.tile([128, NHL, H], f16)
        nc.scalar.dma_start(wo_sb[:], woT.rearrange("(h p) n -> p h n", p=128))
        wq_sb = consts.tile([128, KH, NHL * HD], f16)
        nc.scalar.dma_start(wq_sb[:], wqT.rearrange("(p j) n -> p j n", p=128))
        wk_sb = consts.tile([128, KH, NHL * HD], f16)
        nc.scalar.dma_start(wk_sb[:], wkT.rearrange("(p j) n -> p j n", p=128))
        wv_sb = consts.tile([128, KH, NHL * HD], f16)
        nc.scalar.dma_start(wv_sb[:], wvT.rearrange("(p j) n -> p j n", p=128))

        # flattened per-(b,h,q) column index: j = h*BS + b*S + q (h-major)
        qT_h = stage.tile([128, NHL, BS], f16)
        qT_8 = stage.tile([128, NHL, BS], f8)
        kT_h = stage.tile([128, NHL, BS], f16)
        v_st = stage.tile([S, B, NHL * HD], f16)    # new-token V [t, b, h*HD+d]
        v_sb = stage.tile([BS, NHL * HD], f16)
        en_all = stage.tile([S, BH * S], f16)       # exp of new-token scoresT
        epart = stage.tile([128, BH * S], f32)      # denominator partials
        oT_all = stage.tile([128, BH * S], f32)     # unnormalized outT
        recip = stage.tile([128, BH * S], f32)      # 1/denominator replicated
        oTn = stage.tile([128, BH * S], f16)        # normalized outT (fp16)
        meps_sb = stage.tile([1, BH * HD], f16)     # mean V fp8 residual
        ecr_sb = stage.tile([1, BH * S], f16)       # cache softmax mass row

        for _rep in range(repeat):
            _attention_body(nc, tc, tile, mybir, cache_mask, locals())

    nc.compile()
    return nc


def _attention_body(nc, tc, tile, mybir, cache_mask, env):
    from contextlib import ExitStack

    f32 = mybir.dt.float32
    f16 = mybir.dt.float16
    f8 = mybir.dt.float8e4
    AL = mybir.AluOpType
    AX = mybir.AxisListType
    ACT_EXP = mybir.ActivationFunctionType.Exp
    half = HD // 2

    kv8, meps, outp = env["kv8"], env["meps"], env["outp"]
    cq, sq, ck, sk = env["cq"], env["sq"], env["ck"], env["sk"]
    hT_sb, mN_sb, wo_sb = env["hT_sb"], env["mN_sb"], env["wo_sb"]
    wq_sb, wk_sb, wv_sb = env["wq_sb"], env["wk_sb"], env["wv_sb"]
    mT_sb = env.get("mT_sb")
    ones, ones_h = env["ones"], env["ones_h"]
    qT_h, kT_h, qT_8 = env["qT_h"], env["kT_h"], env["qT_8"]
    v_st, v_sb = env["v_st"], env["v_sb"]
    en_all, epart = env["en_all"], env["epart"]
    oT_all, recip, oTn = env["oT_all"], env["recip"], env["oTn"]
    meps_sb, ecr_sb = env["meps_sb"], env["ecr_sb"]

    if True:
        with ExitStack() as p1:
            kvpool = p1.enter_context(tc.tile_pool(name="kv", bufs=KVLOOK + 2))
            kt_t = [None] * BH
            e_sb = [None] * BH
            po = [None] * BH

            def kv_dma(bh):
                # on the sync ring only: the scalar(ACT) queue runs the exp
                # instructions, whose semaphore waits would stall DMA
                # triggers queued behind them
                tkv = kvpool.tile([128, 2 * NCH, 128], f8, tag="kv8")
                nc.sync.dma_start(
                    tkv[:], kv8[bh].rearrange("p (j d) -> p j d", j=2 * NCH))
                kt_t[bh] = tkv

            # prefetch the first kv tiles before anything else so the DMA
            # rings are never idle during the projection phase
            for bh in range(min(KVLOOK + 1, BH)):
                kv_dma(bh)
            nc.gpsimd.dma_start(meps_sb[:], meps)

            # ---- projections + RoPE ----
            with ExitStack() as ph:
                ppool = ph.enter_context(
                    tc.tile_pool(name="pproj", bufs=1, space="PSUM"))
                tpool = ph.enter_context(tc.tile_pool(name="ropetmp", bufs=2))
                qkpool = ph.enter_context(tc.tile_pool(name="qk32", bufs=2))

                def rope(dst, psrc, cos_t, sin_t):
                    # dst = psrc * cos + shift64(psrc) * sin (sin sign-folded)
                    tmp = tpool.tile([128, BS], f32, tag="ropetmp")
                    nc.vector.tensor_tensor(
                        out=tmp[0:half, :], in0=psrc[half:128, :],
                        in1=sin_t[0:half, :], op=AL.mult)
                    nc.vector.tensor_tensor(
                        out=tmp[half:128, :], in0=psrc[0:half, :],
                        in1=sin_t[half:128, :], op=AL.mult)
                    dst32 = qkpool.tile([128, BS], f32, tag="qk32")
                    nc.vector.tensor_tensor(
                        out=dst32[:], in0=psrc[:], in1=cos_t[:], op=AL.mult)
                    nc.vector.tensor_tensor(
                        out=dst32[:], in0=dst32[:], in1=tmp[:], op=AL.add)
                    nc.vector.tensor_copy(dst, dst32[:])

                for h in range(NHL):
                    pq = ppool.tile([128, BS], f32, tag=f"pq{h}")
                    pk = ppool.tile([128, BS], f32, tag=f"pk{h}")
                    for c in range(KH):
                        nc.tensor.matmul(
                            pq[:], lhsT=wq_sb[:, c, h * HD:(h + 1) * HD],
                            rhs=hT_sb[:, c, :], start=(c == 0),
                            stop=(c == KH - 1))
                    for c in range(KH):
                        nc.tensor.matmul(
                            pk[:], lhsT=wk_sb[:, c, h * HD:(h + 1) * HD],
                            rhs=hT_sb[:, c, :], start=(c == 0),
                            stop=(c == KH - 1))
                    rope(qT_h[:, h, :], pq[:], cq, sq)
                    rope(kT_h[:, h, :], pk[:], ck, sk)
                    nc.vector.tensor_copy(qT_8[:, h, :], qT_h[:, h, :])

                pv = ppool.tile([BS, NHL * HD], f32, tag="pv")
                for c in range(KH):
                    nc.tensor.matmul(
                        pv[:], lhsT=hT_sb[:, c, :], rhs=wv_sb[:, c, :],
                        start=(c == 0), stop=(c == KH - 1))
                nc.vector.tensor_copy(v_sb[:], pv[:])
                for b in range(B):
                    nc.gpsimd.dma_start(v_st[:, b, :],
                                        v_sb[b * S:(b + 1) * S, :])

            # ---- fused attention pass over (b, h) ----
            epool = p1.enter_context(tc.tile_pool(name="e", bufs=3))
            s4pool = p1.enter_context(tc.tile_pool(name="s4", bufs=2))
            npool = p1.enter_context(tc.tile_pool(name="ntmp", bufs=2))
            pspool = p1.enter_context(
                tc.tile_pool(name="psc", bufs=3, space="PSUM"))
            popool = p1.enter_context(
                tc.tile_pool(name="po", bufs=3, space="PSUM"))
            pnpool = p1.enter_context(
                tc.tile_pool(name="psn", bufs=1, space="PSUM"))

            # ---- new-token scores, batched: one PSUM tile, one exp ----
            pn_all = pnpool.tile([S, BH * S], f32, tag="pn")
            for bh in range(BH):
                b, h = divmod(bh, NHL)
                col = h * BS + b * S
                nc.tensor.matmul(
                    pn_all[:, col:col + S],
                    lhsT=kT_h[:, h, b * S:(b + 1) * S],
                    rhs=qT_h[:, h, b * S:(b + 1) * S],
                    start=True, stop=True)
            sn_all = npool.tile([S, BH * S], f32, tag="sn")
            for h in range(NHL):
                nc.vector.tensor_tensor(
                    out=sn_all[:, h * BS:(h + 1) * BS],
                    in0=pn_all[:, h * BS:(h + 1) * BS],
                    in1=mN_sb.rearrange("t b q -> t (b q)"), op=AL.add)
            nc.scalar.activation(en_all[:], sn_all[:], ACT_EXP)

            def scores(bh):
                b, h = divmod(bh, NHL)
                col = h * BS + b * S
                qs8 = qT_8[:, h, b * S:(b + 1) * S]
                # cache scores: 32 stationary-K matmuls into one PSUM tile
                ps = pspool.tile([128, NCH, S], f32, tag="ps")
                for j in range(NCH):
                    nc.tensor.matmul(
                        ps[:, j, :], lhsT=kt_t[bh][:, j, :], rhs=qs8,
                        start=True, stop=True)
                e = epool.tile([128, NCH, S], f16, tag="e")
                if cache_mask:
                    s4 = s4pool.tile([128, NCH * S], f32, tag="s4")
                    nc.vector.tensor_tensor(
                        out=s4[:], in0=ps.rearrange("p j q -> p (j q)"),
                        in1=mT_sb[:, b, :], op=AL.add)
                    nc.scalar.activation(
                        e.rearrange("p j q -> p (j q)"), s4[:], ACT_EXP)
                else:
                    nc.scalar.activation(
                        e.rearrange("p j q -> p (j q)"),
                        ps.rearrange("p j q -> p (j q)"), ACT_EXP)
                e_sb[bh] = e
                nc.vector.reduce_sum(
                    epart[:, col:col + S], e.rearrange("p j q -> p q j"),
                    axis=AX.X)

            def vpass(bh):
                b, h = divmod(bh, NHL)
                col = h * BS + b * S
                p = popool.tile([128, S], f32, tag="po")
                for j in range(NCH):
                    nc.tensor.matmul(
                        p[:], lhsT=kt_t[bh][:, NCH + j, :],
                        rhs=e_sb[bh][:, j, :], start=(j == 0), stop=False)
                nc.tensor.matmul(
                    p[:], lhsT=v_st[:, b, h * HD:(h + 1) * HD],
                    rhs=en_all[:, col:col + S], start=False, stop=True)
                po[bh] = p
                nc.vector.tensor_copy(oT_all[:, col:col + S], p[:])

            for bh in range(BH):
                scores(bh)
                if bh + KVLOOK + 1 < BH:
                    kv_dma(bh + KVLOOK + 1)
                if bh > 0:
                    vpass(bh - 1)
            vpass(BH - 1)

        # ---- denominators + V-residual correction + normalize ----
        with ExitStack() as pd_:
            dpool = pd_.enter_context(
                tc.tile_pool(name="pden", bufs=1, space="PSUM"))
            pdc = dpool.tile([128, BH * S], f32, tag="pdc")
            nc.tensor.matmul(pdc[:], lhsT=ones[:], rhs=epart[:],
                             start=True, stop=True)
            nc.vector.tensor_copy(ecr_sb[:], pdc[0:1, :])
            pcorr = dpool.tile([128, BH * S], f32, tag="pcorr")
            for bh in range(BH):
                b, h = divmod(bh, NHL)
                col = h * BS + b * S
                nc.tensor.matmul(
                    pcorr[:, col:col + S],
                    lhsT=meps_sb[0:1, bh * HD:(bh + 1) * HD],
                    rhs=ecr_sb[0:1, col:col + S], start=True, stop=True)
            pd = dpool.tile([128, BH * S], f32, tag="pd")
            nc.tensor.matmul(pd[:], lhsT=ones[:], rhs=epart[:],
                             start=True, stop=False)
            nc.tensor.matmul(pd[:], lhsT=ones_h[:], rhs=en_all[:],
                             start=False, stop=True)
            nc.vector.reciprocal(recip[:], pd[:])
            nc.vector.tensor_tensor(out=oT_all[:], in0=oT_all[:],
                                    in1=pcorr[:], op=AL.add)
            nc.vector.tensor_tensor(out=oTn[:], in0=oT_all[:],
                                    in1=recip[:], op=AL.mult)

        # ---- o_proj ----
        with ExitStack() as po_:
            opool = po_.enter_context(tc.tile_pool(name="oout", bufs=2))
            oppool = po_.enter_context(
                tc.tile_pool(name="pop", bufs=2, space="PSUM"))
            NBLK = 512
            for nb in range(H // NBLK):
                pout = oppool.tile([BS, NBLK], f32, tag="pout")
                for h in range(NHL):
                    nc.tensor.matmul(
                        pout[:], lhsT=oTn[:, h * BS:(h + 1) * BS],
                        rhs=wo_sb[:, h, nb * NBLK:(nb + 1) * NBLK],
                        start=(h == 0), stop=(h == NHL - 1))
                osb = opool.tile([BS, NBLK], f16, tag="osb")
                nc.vector.tensor_copy(osb[:], pout[:])
                nc.sync.dma_start(outp[:, nb * NBLK:(nb + 1) * NBLK],
                                  osb[:])


def _get_nc(cache_mask=False):
    key = ("nc", cache_mask)
    if key not in _CACHE:
        _CACHE[key] = _build_nc(cache_mask=cache_mask)
    return _CACHE[key]


def _prep_inputs(hidden_states, position_ids, past_key, past_value,
                 attention_mask, Wq, Wk, Wv, Wo):
    """Host-side marshaling: per-core input dicts."""
    f = np.float32
    h16 = np.float16
    hidden = np.asarray(hidden_states, f)
    pos = np.asarray(position_ids)
    pk = np.asarray(past_key, f)
    pv = np.asarray(past_value, f)
    mask = np.asarray(attention_mask, f)
    Wq = np.asarray(Wq, f)
    Wk = np.asarray(Wk, f)
    Wv = np.asarray(Wv, f)
    Wo = np.asarray(Wo, f)

    hT = np.ascontiguousarray(hidden.reshape(BS, H).T.astype(h16))

    posf = pos.reshape(BS).astype(f)
    inv_freq = (1.0 / (ROPE_BASE ** (np.arange(0, HD, 2, dtype=f) / HD))).astype(f)
    ang = posf[:, None] * inv_freq[None, :]          # [BS, 64]
    cos = np.cos(ang).astype(f).T                    # [64, BS]
    sin = np.sin(ang).astype(f).T
    cos_full = np.concatenate([cos, cos], axis=0)    # [128, BS]
    sin_fold = np.concatenate([-sin, sin], axis=0)   # sign-folded
    scale = f(1.0 / np.sqrt(HD))
    cosq = np.ascontiguousarray(cos_full * scale)
    sinq = np.ascontiguousarray(sin_fold * scale)
    cosk = np.ascontiguousarray(cos_full)
    sink = np.ascontiguousarray(sin_fold)

    m = mask[:, 0]                                   # [B, S, TOTAL]
    mc = m[:, :, :CACHE]                             # [B, S, CACHE]
    # kernel token order: cache token t = 32*p + j -> maskT[p, b, j, q]
    maskT = np.ascontiguousarray(
        mc.reshape(B, S, 128, NCH).transpose(2, 0, 3, 1))
    mn = m[:, :, CACHE:]                             # [B, S, S]
    maskN = np.ascontiguousarray(mn.transpose(2, 0, 1).reshape(S, B * S))

    # packed fp8 KV: [BH, 128, 8192] per core; K half is K^T in the
    # permuted token order (chunk j, row p <-> token 32p + j), V half is V
    # in natural (p, j, d) blocks = same permuted order.
    pk5 = pk.reshape(B, NH, 128, NCH, HD)
    pv5 = pv.reshape(B, NH, 128, NCH, HD)

    in_maps = []
    from concourse import mybir as _mybir
    f8np = _mybir.dt.np(_mybir.dt.float8e4)
    for core in range(NCORES):
        h0 = core * NHL
        rows = slice(h0 * HD, (h0 + NHL) * HD)
        ktp = pk5[:, h0:h0 + NHL].transpose(0, 1, 4, 3, 2).astype(f8np)
        vp8 = pv5[:, h0:h0 + NHL].astype(f8np)
        # mean fp8 residual of V over the 4096 cache tokens, per (bh, d)
        meps = (pv5[:, h0:h0 + NHL]
                - vp8.astype(f)).mean(axis=(2, 3)).astype(h16)
        kvp = np.concatenate(
            [ktp.reshape(BH, 128, NCH * HD),
             vp8.reshape(BH, 128, NCH * HD)], axis=2)
        in_maps.append({
            "hT": hT,
            "wqT": np.ascontiguousarray(Wq[rows, :].T.astype(h16)),
            "wkT": np.ascontiguousarray(Wk[rows, :].T.astype(h16)),
            "wvT": np.ascontiguousarray(Wv[rows, :].T.astype(h16)),
            "woT": np.ascontiguousarray(Wo[:, rows].T.astype(h16)),
            "kv8": np.ascontiguousarray(kvp),
            "meps": np.ascontiguousarray(meps.reshape(1, BH * HD)),
            "maskT": maskT,
            "maskN": maskN,
            "cosq": cosq, "sinq": sinq, "cosk": cosk, "sink": sink,
        })
    return in_maps


def kernel(**inputs):
    from concourse.bass_utils import run_bass_kernel_spmd

    # The cache-region mask is structurally zero for this module (causal mask
    # over tokens that all precede the new ones). Only build the general
    # masked variant if the input actually carries nonzero cache-mask values.
    mc = np.asarray(inputs["attention_mask"], np.float32)[:, 0, :, :CACHE]
    nc = _get_nc(cache_mask=bool(np.any(mc != 0.0)))
    in_maps = _prep_inputs(**inputs)
    res = run_bass_kernel_spmd(nc, in_maps, list(range(NCORES)), trace=False)
    out = np.zeros((BS, H), np.float32)
    for r in res.results:
        out += np.asarray(r["outp"], np.float32)
    return out.reshape(B, S, H)
